# revision 1
# baseline (speedup 1.0000x reference)
"""CoSen cross-entropy loss kernel for Trainium2 (8 NeuronCores, data-parallel).

Math note: the reference computes
    m_i   = xi[label_i, argmax_j x_ij]
    denom = log(sum_j m_i * exp(x_ij)) = log(m_i) + logsumexp(x_i)
    log_s = log(m_i) + x - denom = x - logsumexp(x_i)
so m (and therefore xi and the argmax) cancels exactly and the loss is plain
cross-entropy:  nll = mean_i( logsumexp(x_i) - x[i, label_i] ).

Device strategy (per core, 4096 rows x 1000 cols):
  - host casts scores to fp16 (halves HBM traffic; ~1e-7 rel effect on loss)
  - stream x in [128, 8, 1000] gather-group tiles, two 1 MB HWDGE DMAs each
  - per 128-row block, sum-of-exp runs on one of three engines (pattern):
      'A' ScalarE table exp + fused accum (exact)
      'D' VectorE Schraudolph int16-bitcast exp, VectorE reduce
      'P' GpSimd Schraudolph convert, VectorE reduce
  - GpSimd ap_gather picks the fp16 pairs containing x[p,label]; a host-built
    diagonal+parity mask and a tiny mul+reduce extract x[i,label_i]
  - tail: Ln, reduce, subtract -> per-partition partial sums [128,1]
  - host: sum 8x128 partials / B
"""

import os as _os
import sys

import numpy as np

if "/opt/trn_rl_repo" not in sys.path:
    sys.path.insert(0, "/opt/trn_rl_repo")

B = 32768
C = 1000
NCORES = 8
RPC = B // NCORES          # rows per core = 4096
P = 128                    # partitions
NBLK = RPC // P            # 32 blocks of 128 rows per core
GPB = int(_os.environ.get("GPB", "16"))  # blocks per gather group
NG = NBLK // GPB           # 4 gather groups
DPB = int(_os.environ.get("DPB", "8"))  # blocks per DMA (2 MB contiguous fp16)
GW = GPB * 16 * 2          # gather output width per group (fp16 pairs)

# Engine assignment pattern for the 32 blocks' sum-of-exp, repeated cyclically.
BLOCK_PATTERN = _os.environ.get("BLOCK_PATTERN", "ADDAADDA")
SKIP_GATHER = _os.environ.get("SKIP_GATHER", "0") == "1"
# Host pre-transpose: DRAM layout [g][p][b][c] makes each partition's DMA a
# single long contiguous run (GPB*2000B) instead of strided 2000B segments.
XLAYOUT = _os.environ.get("XLAYOUT", "1") == "1"

# fp16 Schraudolph exp: bitcast16(round(A16*x + B16)) ~ exp(x). c calibrated
# so mean relative error over uniform mantissa positions is ~zero (residual
# lse bias ~8e-4 on the blocks that use it).
_SCHRAUDOLPH_C = 0.05640058203281112
A16 = float(np.float32(2**10 / np.log(2)))
B16 = float(np.float32((15 - _SCHRAUDOLPH_C) * 2**10))

# Tail log via bitcast: ln(s) ~ (bitcast_i32(s)*2^-23 - (127 - c2)) * ln2,
# c2 = E_f[log2(1+f) - f] zeroes the mean error. Avoids the Ln act-table load.
LOG_APPROX = _os.environ.get("LOG_APPROX", "0") == "1"
_C2 = 0.0573049591429322  # mean of log2(1+f)-f over f~U[0,1)
LG_A = float(np.float32(np.log(2) / 2**23))
LG_B = float(np.float32(-(127 - _C2) * np.log(2)))

_CACHE = {}


def build_nc(repeat=1, loop=1):
    import contextlib

    import concourse.bacc as bacc
    import concourse.tile as tile
    from concourse import mybir

    nc = bacc.Bacc("TRN2", target_bir_lowering=False, debug=False, num_devices=NCORES)

    x = nc.dram_tensor("x", [RPC, C], mybir.dt.float16, kind="ExternalInput").ap()
    idx = nc.dram_tensor("idx", [P, NBLK], mybir.dt.int16, kind="ExternalInput").ap()
    dmask = nc.dram_tensor(
        "dmask", [P, NG * GW], mybir.dt.float8e4, kind="ExternalInput"
    ).ap()
    out = nc.dram_tensor("out", [P, 1], mybir.dt.float32, kind="ExternalOutput").ap()

    # row (g*GPB + b)*128 + p  ->  group g, sbuf [p, b, c]
    if XLAYOUT:
        x_r = x.rearrange("(g p b) c -> g p b c", p=P, b=GPB)
    else:
        x_r = x.rearrange("(g b p) c -> g p b c", p=P, b=GPB)

    with tile.TileContext(nc) as tc:
        with (
            tc.tile_pool(name="big", bufs=int(_os.environ.get("BIGBUFS", "2"))) as big_pool,
            tc.tile_pool(name="scratch", bufs=2) as scratch,
            tc.tile_pool(name="small", bufs=1) as small,
        ):
            idx_sb = small.tile([P, NBLK], mybir.dt.int16)
            nc.gpsimd.dma_start(out=idx_sb[:], in_=idx)
            mask_sb = small.tile([P, NG * GW], mybir.dt.float8e4)
            nc.gpsimd.dma_start(out=mask_sb[:], in_=dmask)
            s_all = small.tile([P, NBLK], mybir.dt.float32)
            xlab_all = small.tile([P, NG], mybir.dt.float32)
            if SKIP_GATHER:
                nc.vector.memset(xlab_all[:], 0.0)

            loop_cm = tc.For_i(0, loop, 1) if loop > 1 else contextlib.nullcontext()
            with loop_cm:
                for i, g in enumerate(
                    [g for _ in range(repeat) for g in range(NG)]
                ):
                    xt = big_pool.tile([P, GPB, C], mybir.dt.float16, tag="xt")
                    if i == 0:
                        # small leading chunks so compute starts sooner
                        splits = [0, 1, 2, 4]
                        while splits[-1] < GPB:
                            splits.append(min(splits[-1] + DPB, GPB))
                    else:
                        splits = list(range(0, GPB + 1, DPB))
                    for lo, hi in zip(splits[:-1], splits[1:]):
                        nc.sync.dma_start(
                            out=xt[:, lo:hi, :],
                            in_=x_r[g, :, lo:hi, :],
                        )

                    for b in range(GPB):
                        k = g * GPB + b
                        kind = BLOCK_PATTERN[k % len(BLOCK_PATTERN)]
                        if kind == "A":
                            e = scratch.tile([P, C], mybir.dt.float16, tag="e")
                            nc.scalar.activation(
                                out=e[:],
                                in_=xt[:, b, :],
                                func=mybir.ActivationFunctionType.Exp,
                                accum_out=s_all[:, k : k + 1],
                            )
                        else:
                            eng = nc.vector if kind == "D" else nc.gpsimd
                            i16 = scratch.tile(
                                [P, C], mybir.dt.int16, tag=f"i16{kind}"
                            )
                            eng.tensor_scalar(
                                out=i16[:],
                                in0=xt[:, b, :],
                                scalar1=A16,
                                scalar2=B16,
                                op0=mybir.AluOpType.mult,
                                op1=mybir.AluOpType.add,
                            )
                            nc.vector.tensor_reduce(
                                out=s_all[:, k : k + 1],
                                in_=i16[:].bitcast(mybir.dt.float16),
                                axis=mybir.AxisListType.X,
                                op=mybir.AluOpType.add,
                            )

                    if not SKIP_GATHER:
                        # gather fp16 pairs: elem (b*C+l) is in pair (b*C+l)//2
                        gt = scratch.tile([P, GW], mybir.dt.float16, tag="gt")
                        nc.gpsimd.ap_gather(
                            out_ap=gt[:],
                            in_ap=xt[:].rearrange("p b c -> p (b c)"),
                            idxs_ap=idx_sb[:, g * GPB : (g + 1) * GPB],
                            channels=P,
                            num_elems=GPB * C // 2,
                            d=2,
                            num_idxs=GPB * 16,
                        )
                        junk = scratch.tile([P, GW], mybir.dt.float16, tag="junk")
                        nc.vector.tensor_mul(
                            junk[:], gt[:], mask_sb[:, g * GW : (g + 1) * GW]
                        )
                        nc.vector.tensor_reduce(
                            out=xlab_all[:, g : g + 1],
                            in_=junk[:],
                            axis=mybir.AxisListType.X,
                            op=mybir.AluOpType.add,
                        )

            lse = small.tile([P, NBLK], mybir.dt.float32)
            if LOG_APPROX:
                nc.vector.tensor_scalar(
                    out=lse[:],
                    in0=s_all[:].bitcast(mybir.dt.int32),
                    scalar1=LG_A,
                    scalar2=LG_B,
                    op0=mybir.AluOpType.mult,
                    op1=mybir.AluOpType.add,
                )
            else:
                nc.scalar.activation(
                    out=lse[:], in_=s_all[:], func=mybir.ActivationFunctionType.Ln
                )
            lse_sum = small.tile([P, 1], mybir.dt.float32)
            nc.vector.tensor_reduce(
                out=lse_sum[:], in_=lse[:], axis=mybir.AxisListType.X,
                op=mybir.AluOpType.add,
            )
            xl_sum = small.tile([P, 1], mybir.dt.float32)
            nc.vector.tensor_reduce(
                out=xl_sum[:], in_=xlab_all[:], axis=mybir.AxisListType.X,
                op=mybir.AluOpType.add,
            )
            part = small.tile([P, 1], mybir.dt.float32)
            nc.vector.tensor_sub(part[:], lse_sum[:], xl_sum[:])
            nc.sync.dma_start(out=out, in_=part[:])

    nc.compile()
    return nc


def make_inputs(cls_score, label):
    """Host-side sharding: per-core fp16 x slice, pair-gather indices, and the
    diagonal+parity extraction mask."""
    cls_score = np.asarray(cls_score, dtype=np.float32)
    label = np.asarray(label).astype(np.int64)
    assert cls_score.shape == (B, C), cls_score.shape
    assert label.shape == (B,), label.shape
    x16 = cls_score.astype(np.float16)

    q = np.arange(GPB * 16) % 16          # group slot of each gather position
    pm = np.arange(P) % 16                # this partition's slot
    diag = q[None, :] == pm[:, None]      # [P, GPB*16]

    in_maps = []
    for c in range(NCORES):
        xc = x16[c * RPC : (c + 1) * RPC]
        if XLAYOUT:
            xc = (
                xc.reshape(NG, GPB, P, C)
                .transpose(0, 2, 1, 3)
                .reshape(RPC, C)
            )
        xc = np.ascontiguousarray(xc)
        lab = label[c * RPC : (c + 1) * RPC].reshape(NG, GPB, P)
        flat = np.arange(GPB)[None, :, None] * C + lab      # [NG, GPB, P]
        # pair index within the group; layout [P, NG*GPB]
        idx = np.ascontiguousarray(
            (flat // 2).transpose(2, 0, 1).reshape(P, NG * GPB).astype(np.int16)
        )
        # dmask[p, g*GW + (b*16+q)*2 + r] = diag and r == parity(label[g,b,p])
        par = (flat % 2).transpose(2, 0, 1)                 # [P, NG, GPB]
        par_b = np.repeat(par[:, :, :, None], 16, axis=3).reshape(P, NG, GPB * 16)
        r = np.arange(2)[None, None, None, :]
        import ml_dtypes
        m = (diag[:, None, :, None] & (r == par_b[:, :, :, None])).astype(
            ml_dtypes.float8_e4m3
        )
        dmask = np.ascontiguousarray(m.reshape(P, NG * GW))
        in_maps.append({"x": xc, "idx": idx, "dmask": dmask})
    return in_maps


def _run(cls_score, label, **spmd_kwargs):
    import time

    from concourse.bass_utils import run_bass_kernel_spmd

    if "nc" not in _CACHE:
        _CACHE["nc"] = build_nc()
    nc = _CACHE["nc"]

    in_maps = make_inputs(cls_score, label)
    last_err = None
    for attempt in range(4):
        try:
            res = run_bass_kernel_spmd(
                nc, in_maps, core_ids=list(range(NCORES)), **spmd_kwargs
            )
            break
        except Exception as e:  # transient device-unrecoverable states heal
            last_err = e
            time.sleep(10 * (attempt + 1))
    else:
        raise last_err
    total = np.float64(0.0)
    for r in res.results:
        total += r["out"].astype(np.float64).sum()
    return np.float32(total / B), res


def kernel(cls_score, label, xi=None, **_ignored):
    return _run(cls_score, label)[0]


if __name__ == "__main__":
    rng = np.random.default_rng(0)
    x = rng.standard_normal((B, C), dtype=np.float32)
    lab = rng.integers(0, C, size=(B,)).astype(np.int64)
    got = kernel(x, lab, np.ones((C, C), np.float32))
    m = x.max(axis=-1, keepdims=True)
    lse = (np.log(np.exp(x - m).sum(-1)) + m[:, 0]).astype(np.float64)
    want = (lse - x[np.arange(B), lab]).mean()
    print("kernel:", got, "ref:", want, "rel:", abs(got - want) / abs(want))

